# revision 40
# baseline (speedup 1.0000x reference)
import sys

sys.path.insert(0, "/opt/trn_rl_repo")

import numpy as np
import ml_dtypes

import concourse.bass as bass
import concourse.tile as tile
from concourse import bacc, mybir
from concourse.bass_utils import run_bass_kernel_spmd

# Problem constants (hardcoded per contract)
B, N, F = 8, 512, 16
D, PH, PW = 150, 26, 26
PWP = 28  # patch cols padded to 28 (parity shift keeps scatter base even)
IMG = 128
HP = IMG + 2 * PH  # 180 padded canvas rows
WP = IMG + 2 * PW  # 180 padded canvas cols
CSTRIDE = 184  # canvas row stride in SBUF (padded, even)
HWP = PH * PWP  # 728 padded patch size
C = 64  # spline coefficients per voxel
GRP = 128  # groups per core
EPG = 4  # emitters per group
EW = 32  # partition rows per emitter (F=16 + 16 zero pad, 32-aligned)
K = EPG * C  # 256 contraction (block diagonal)
KC = K // 128  # 2 K-chunks
ROW0 = 14  # smallest y_idx given input coordinate ranges
VROWS = 166 - ROW0  # canvas rows [14, 165]
RPB = 16  # offset registers loaded per batched reg_load
PWW = 27  # scatter window width: parity-shifted content spans cols 0..26

# gpsimd tensor_tensor/regs crash the runtime on this stack — DVE-only scatter
G_GP = 0

_compiled = None


def _build_bass():
    nc = bacc.Bacc()
    f32 = mybir.dt.float32
    bf16 = mybir.dt.bfloat16
    i32 = mybir.dt.int32

    lhsT_d = nc.declare_dram_parameter("lhsT", [GRP, K, 128], bf16, isOutput=False)
    rhs_d = nc.declare_dram_parameter("rhs", [GRP, K, HWP], bf16, isOutput=False)
    offs_d = nc.declare_dram_parameter("offs", [1, N], i32, isOutput=False)
    zero_d = nc.declare_dram_parameter(
        "zero", [EW, (VROWS + 1) * CSTRIDE], bf16, isOutput=False
    )
    out_d = nc.declare_dram_parameter("out", [F, IMG * IMG], bf16, isOutput=True)

    with tile.TileContext(nc) as tc:
        with (
            tc.tile_pool(name="canvas", bufs=1) as canvas_pool,
            tc.tile_pool(name="weights", bufs=4) as w_pool,
            tc.tile_pool(name="slabs", bufs=4) as s_pool,
            tc.tile_pool(name="psum", bufs=3, space="PSUM") as p_pool,
            tc.tile_pool(name="small", bufs=1) as small_pool,
        ):
            # +1 scratch row: the flat ds() slice of a patch claims PH full
            # rows even though only PWP cols of the last row are touched
            # offs first so register loads aren't stuck behind the zero DMA
            offs_t = small_pool.tile([1, N], i32)
            nc.sync.dma_start(offs_t[:], offs_d[:])

            canvas_v = canvas_pool.tile([EW, (VROWS + 1) * CSTRIDE], bf16, tag="cv")
            # zero half on the (idle-at-startup) DVE, half via DMA — halves
            # the startup DMA-bandwidth contention with group prefetch
            split_c = 16896  # ~60% on DVE, rest via DMA
            nc.vector.memset(canvas_v[:, 0:split_c].bitcast(i32), 0)
            nc.sync.dma_start(canvas_v[:, split_c:], zero_d[:, split_c:])
            regs_v = [nc.vector.alloc_register(f"offv{i}") for i in range(RPB)]

            for g in range(GRP):
                lt = w_pool.tile([128, KC * 128], bf16, tag="lt")
                nc.sync.dma_start(
                    lt[:].rearrange("p (kc m) -> p kc m", kc=KC),
                    lhsT_d[g].rearrange("(kc k) m -> k kc m", k=128),
                )
                rt = s_pool.tile([128, KC * HWP], bf16, tag="rt")
                nc.sync.dma_start(
                    rt[:].rearrange("p (kc n) -> p kc n", kc=KC),
                    rhs_d[g].rearrange("(kc k) n -> k kc n", k=128),
                )
                ps = p_pool.tile([128, HWP], f32, tag="ps")
                for kc in range(KC):
                    for n0, n1 in ((0, 512), (512, HWP)):
                        nc.tensor.matmul(
                            ps[:, n0:n1],
                            lhsT=lt[:, kc * 128 : (kc + 1) * 128],
                            rhs=rt[:, kc * HWP + n0 : kc * HWP + n1],
                            start=(kc == 0),
                            stop=(kc == KC - 1),
                        )
                sbp = s_pool.tile([128, HWP], bf16, tag="sbp")
                nc.scalar.copy(out=sbp[:], in_=ps[:])
                ps3 = sbp[:].rearrange("p (h w) -> p h w", h=PH, w=PWP)
                eng, regs, cnv = nc.vector, regs_v, canvas_v
                for i in range(EPG):
                    e = g * EPG + i
                    r = e % RPB
                    if r == 0:
                        eng.reg_load(regs, offs_t[0:1, e : e + RPB])
                    off = eng.snap(
                        regs[r],
                        donate=True,
                        min_val=0,
                        max_val=(VROWS - PH) * CSTRIDE + (WP - PWP),
                    )
                    dst = (
                        cnv[:, bass.ds(off, PH * CSTRIDE)]
                        .rearrange("p (h w) -> p h w", h=PH)[:, :, 0:PWW]
                    )
                    eng.tensor_tensor(
                        out=dst,
                        in0=dst,
                        in1=ps3[EW * i : EW * (i + 1), :, 0:PWW],
                        op=mybir.AluOpType.add,
                    )

            canvas3 = canvas_v[:, 0 : VROWS * CSTRIDE].rearrange(
                "p (h w) -> p h w", h=VROWS, w=CSTRIDE
            )
            out3 = out_d[:].rearrange("p (h w) -> p h w", h=IMG, w=IMG)
            # 4 split DMAs, issued from two engines so descriptor generation
            # for the output tail runs in parallel
            for q in range(4):
                r0, r1 = q * (IMG // 4), (q + 1) * (IMG // 4)
                (nc.sync if q < 2 else nc.scalar).dma_start(
                    out3[:, r0:r1],
                    canvas3[0:F, PH - ROW0 + r0 : PH - ROW0 + r1, PW : PW + IMG],
                )
    if not nc.is_finalized():
        nc.finalize()
    return nc


def _host_prep(xyz, n_photons, coeffs, inv_voxel_size, psf_center):
    """Per-batch host prep: indices, series, photon-folded lhsT, gathered
    parity-shifted rhs, even scatter offsets."""
    u = xyz * inv_voxel_size  # (B,N,3)
    u = u.copy()
    u[..., :2] -= psf_center[:2]
    u[..., 2] += psf_center[2]
    u_floor = np.floor(u)
    frac = u - u_floor
    ui = u_floor.astype(np.int32)
    x_idx = ui[..., 0] + PW  # (B,N)
    y_idx = ui[..., 1] + PH
    z_idx = ui[..., 2]
    frac[..., :2] = 1.0 - frac[..., :2]

    # 64-term series: series[b,n,c], c = kz*16 + kx*4 + ky
    p = frac[..., None] ** np.arange(4, dtype=np.float32)  # (B,N,3,4)
    vx, vy, vz = p[..., 0, :], p[..., 1, :], p[..., 2, :]
    series = (
        vz[..., :, None, None] * vx[..., None, :, None] * vy[..., None, None, :]
    ).reshape(B, N, C)

    series16 = n_photons[..., None] * series[:, :, None, :]  # (B,N,F,C)

    # lhsT[b,g,(slot,c),(i,f)] block diagonal; 32-col blocks per emitter
    lhsT = np.zeros((B, GRP, K, 128), dtype=np.float32)
    s16g = series16.reshape(B, GRP, EPG, F, C)
    for i in range(EPG):
        lhsT[:, :, i * C : (i + 1) * C, i * EW : i * EW + F] = s16g[
            :, :, i
        ].transpose(0, 1, 3, 2)
    lhsT = lhsT.astype(ml_dtypes.bfloat16)

    # two parity-shifted slab variants: slab28[s, z, c, h*28 + s + w]
    co = coeffs.reshape(D, PH, PW, C).transpose(0, 3, 1, 2)  # (D, C, PH, PW)
    slab28 = np.zeros((2, D, C, PH, PWP), dtype=ml_dtypes.bfloat16)
    for s in (0, 1):
        slab28[s, :, :, :, s : s + PW] = co
    slab28 = slab28.reshape(2, D, C, HWP)

    s_par = (x_idx & 1).astype(np.int64)  # (B,N) parity shift
    rhs = slab28[s_par.reshape(-1), z_idx.reshape(-1)].reshape(B, GRP, K, HWP)

    offs = ((y_idx - ROW0) * CSTRIDE + x_idx - s_par).astype(np.int32)  # even
    assert (offs % 2 == 0).all() and (offs >= 0).all()
    assert (offs <= (VROWS - PH) * CSTRIDE + (WP - PWP)).all()
    return lhsT, rhs, offs


def kernel(xyz, n_photons, coeffs, inv_voxel_size, psf_center, img_size):
    global _compiled
    xyz = np.asarray(xyz, dtype=np.float32)
    n_photons = np.asarray(n_photons, dtype=np.float32)
    coeffs = np.asarray(coeffs, dtype=np.float32)
    inv_voxel_size = np.asarray(inv_voxel_size, dtype=np.float32)
    psf_center = np.asarray(psf_center, dtype=np.float32)

    lhsT, rhs, offs = _host_prep(
        xyz, n_photons, coeffs, inv_voxel_size, psf_center
    )

    if _compiled is None:
        _compiled = _build_bass()
    nc = _compiled

    zero = np.zeros((EW, (VROWS + 1) * CSTRIDE), dtype=ml_dtypes.bfloat16)
    in_maps = [
        {"lhsT": lhsT[b], "rhs": rhs[b], "offs": offs[b : b + 1], "zero": zero}
        for b in range(B)
    ]
    res = run_bass_kernel_spmd(nc, in_maps, core_ids=list(range(B)))
    out = np.stack(
        [
            res.results[b]["out"].astype(np.float32).reshape(F, IMG, IMG)
            for b in range(B)
        ],
        axis=0,
    )
    return out


# revision 42
# speedup vs baseline: 1.0192x; 1.0192x over previous
import sys

sys.path.insert(0, "/opt/trn_rl_repo")

import numpy as np
import ml_dtypes

import concourse.bass as bass
import concourse.tile as tile
from concourse import bacc, mybir
from concourse.bass_utils import run_bass_kernel_spmd

# Problem constants (hardcoded per contract)
B, N, F = 8, 512, 16
D, PH, PW = 150, 26, 26
PWP = 28  # patch cols padded to 28 (parity shift keeps scatter base even)
IMG = 128
HP = IMG + 2 * PH  # 180 padded canvas rows
WP = IMG + 2 * PW  # 180 padded canvas cols
CSTRIDE = 184  # canvas row stride in SBUF (padded, even)
HWP = PH * PWP  # 728 padded patch size
C = 64  # spline coefficients per voxel
GRP = 128  # groups per core
EPG = 4  # emitters per group
EW = 32  # partition rows per emitter (F=16 + 16 zero pad, 32-aligned)
K = EPG * C  # 256 contraction (block diagonal)
KC = K // 128  # 2 K-chunks
ROW0 = 14  # smallest y_idx given input coordinate ranges
VROWS = 166 - ROW0  # canvas rows [14, 165]
RPB = 16  # offset registers loaded per batched reg_load

# gpsimd tensor_tensor/regs crash the runtime on this stack — DVE-only scatter
G_GP = 0

_compiled = None


def _build_bass():
    nc = bacc.Bacc()
    f32 = mybir.dt.float32
    bf16 = mybir.dt.bfloat16
    i32 = mybir.dt.int32

    lhsT_d = nc.declare_dram_parameter("lhsT", [GRP, K, 128], bf16, isOutput=False)
    rhs_d = nc.declare_dram_parameter("rhs", [GRP, K, HWP], bf16, isOutput=False)
    offs_d = nc.declare_dram_parameter("offs", [1, N], i32, isOutput=False)
    zero_d = nc.declare_dram_parameter(
        "zero", [EW, (VROWS + 1) * CSTRIDE], bf16, isOutput=False
    )
    out_d = nc.declare_dram_parameter("out", [F, IMG * IMG], bf16, isOutput=True)

    with tile.TileContext(nc) as tc:
        with (
            tc.tile_pool(name="canvas", bufs=1) as canvas_pool,
            tc.tile_pool(name="weights", bufs=4) as w_pool,
            tc.tile_pool(name="slabs", bufs=4) as s_pool,
            tc.tile_pool(name="psum", bufs=3, space="PSUM") as p_pool,
            tc.tile_pool(name="small", bufs=1) as small_pool,
        ):
            # +1 scratch row: the flat ds() slice of a patch claims PH full
            # rows even though only PWP cols of the last row are touched
            # offs first so register loads aren't stuck behind the zero DMA
            offs_t = small_pool.tile([1, N], i32)
            nc.sync.dma_start(offs_t[:], offs_d[:])

            canvas_v = canvas_pool.tile([EW, (VROWS + 1) * CSTRIDE], bf16, tag="cv")
            # zero half on the (idle-at-startup) DVE, half via DMA — halves
            # the startup DMA-bandwidth contention with group prefetch
            split_c = 16896  # ~60% on DVE (idle at startup), rest via DMA
            nc.vector.memset(canvas_v[:, 0:split_c].bitcast(i32), 0)
            nc.sync.dma_start(canvas_v[:, split_c:], zero_d[:, split_c:])
            regs_v = [nc.vector.alloc_register(f"offv{i}") for i in range(RPB)]

            for g in range(GRP):
                lt = w_pool.tile([128, KC * 128], bf16, tag="lt")
                nc.sync.dma_start(
                    lt[:].rearrange("p (kc m) -> p kc m", kc=KC),
                    lhsT_d[g].rearrange("(kc k) m -> k kc m", k=128),
                )
                rt = s_pool.tile([128, KC * HWP], bf16, tag="rt")
                nc.sync.dma_start(
                    rt[:].rearrange("p (kc n) -> p kc n", kc=KC),
                    rhs_d[g].rearrange("(kc k) n -> k kc n", k=128),
                )
                ps = p_pool.tile([128, HWP], f32, tag="ps")
                for kc in range(KC):
                    for n0, n1 in ((0, 512), (512, HWP)):
                        nc.tensor.matmul(
                            ps[:, n0:n1],
                            lhsT=lt[:, kc * 128 : (kc + 1) * 128],
                            rhs=rt[:, kc * HWP + n0 : kc * HWP + n1],
                            start=(kc == 0),
                            stop=(kc == KC - 1),
                        )
                sbp = s_pool.tile([128, HWP], bf16, tag="sbp")
                nc.scalar.copy(out=sbp[:], in_=ps[:])
                ps3 = sbp[:].rearrange("p (h w) -> p h w", h=PH, w=PWP)
                eng, regs, cnv = nc.vector, regs_v, canvas_v
                for i in range(EPG):
                    e = g * EPG + i
                    r = e % RPB
                    if r == 0:
                        eng.reg_load(regs, offs_t[0:1, e : e + RPB])
                    off = eng.snap(
                        regs[r],
                        donate=True,
                        min_val=0,
                        max_val=(VROWS - PH) * CSTRIDE + (WP - PWP),
                    )
                    dst = (
                        cnv[:, bass.ds(off, PH * CSTRIDE)]
                        .rearrange("p (h w) -> p h w", h=PH)[:, :, 0:PWP]
                    )
                    eng.tensor_tensor(
                        out=dst,
                        in0=dst,
                        in1=ps3[EW * i : EW * (i + 1)],
                        op=mybir.AluOpType.add,
                    )

            canvas3 = canvas_v[:, 0 : VROWS * CSTRIDE].rearrange(
                "p (h w) -> p h w", h=VROWS, w=CSTRIDE
            )
            out3 = out_d[:].rearrange("p (h w) -> p h w", h=IMG, w=IMG)
            # 4 split DMAs, issued from two engines so descriptor generation
            # for the output tail runs in parallel
            for q in range(4):
                r0, r1 = q * (IMG // 4), (q + 1) * (IMG // 4)
                (nc.sync if q < 2 else nc.scalar).dma_start(
                    out3[:, r0:r1],
                    canvas3[0:F, PH - ROW0 + r0 : PH - ROW0 + r1, PW : PW + IMG],
                )
    if not nc.is_finalized():
        nc.finalize()
    return nc


def _host_prep(xyz, n_photons, coeffs, inv_voxel_size, psf_center):
    """Per-batch host prep: indices, series, photon-folded lhsT, gathered
    parity-shifted rhs, even scatter offsets."""
    u = xyz * inv_voxel_size  # (B,N,3)
    u = u.copy()
    u[..., :2] -= psf_center[:2]
    u[..., 2] += psf_center[2]
    u_floor = np.floor(u)
    frac = u - u_floor
    ui = u_floor.astype(np.int32)
    x_idx = ui[..., 0] + PW  # (B,N)
    y_idx = ui[..., 1] + PH
    z_idx = ui[..., 2]
    frac[..., :2] = 1.0 - frac[..., :2]

    # 64-term series: series[b,n,c], c = kz*16 + kx*4 + ky
    p = frac[..., None] ** np.arange(4, dtype=np.float32)  # (B,N,3,4)
    vx, vy, vz = p[..., 0, :], p[..., 1, :], p[..., 2, :]
    series = (
        vz[..., :, None, None] * vx[..., None, :, None] * vy[..., None, None, :]
    ).reshape(B, N, C)

    series16 = n_photons[..., None] * series[:, :, None, :]  # (B,N,F,C)

    # lhsT[b,g,(slot,c),(i,f)] block diagonal; 32-col blocks per emitter
    lhsT = np.zeros((B, GRP, K, 128), dtype=np.float32)
    s16g = series16.reshape(B, GRP, EPG, F, C)
    for i in range(EPG):
        lhsT[:, :, i * C : (i + 1) * C, i * EW : i * EW + F] = s16g[
            :, :, i
        ].transpose(0, 1, 3, 2)
    lhsT = lhsT.astype(ml_dtypes.bfloat16)

    # two parity-shifted slab variants: slab28[s, z, c, h*28 + s + w]
    co = coeffs.reshape(D, PH, PW, C).transpose(0, 3, 1, 2)  # (D, C, PH, PW)
    slab28 = np.zeros((2, D, C, PH, PWP), dtype=ml_dtypes.bfloat16)
    for s in (0, 1):
        slab28[s, :, :, :, s : s + PW] = co
    slab28 = slab28.reshape(2, D, C, HWP)

    s_par = (x_idx & 1).astype(np.int64)  # (B,N) parity shift
    rhs = slab28[s_par.reshape(-1), z_idx.reshape(-1)].reshape(B, GRP, K, HWP)

    offs = ((y_idx - ROW0) * CSTRIDE + x_idx - s_par).astype(np.int32)  # even
    assert (offs % 2 == 0).all() and (offs >= 0).all()
    assert (offs <= (VROWS - PH) * CSTRIDE + (WP - PWP)).all()
    return lhsT, rhs, offs


def kernel(xyz, n_photons, coeffs, inv_voxel_size, psf_center, img_size):
    global _compiled
    xyz = np.asarray(xyz, dtype=np.float32)
    n_photons = np.asarray(n_photons, dtype=np.float32)
    coeffs = np.asarray(coeffs, dtype=np.float32)
    inv_voxel_size = np.asarray(inv_voxel_size, dtype=np.float32)
    psf_center = np.asarray(psf_center, dtype=np.float32)

    lhsT, rhs, offs = _host_prep(
        xyz, n_photons, coeffs, inv_voxel_size, psf_center
    )

    if _compiled is None:
        _compiled = _build_bass()
    nc = _compiled

    zero = np.zeros((EW, (VROWS + 1) * CSTRIDE), dtype=ml_dtypes.bfloat16)
    in_maps = [
        {"lhsT": lhsT[b], "rhs": rhs[b], "offs": offs[b : b + 1], "zero": zero}
        for b in range(B)
    ]
    res = run_bass_kernel_spmd(nc, in_maps, core_ids=list(range(B)))
    out = np.stack(
        [
            res.results[b]["out"].astype(np.float32).reshape(F, IMG, IMG)
            for b in range(B)
        ],
        axis=0,
    )
    return out
